# revision 23
# baseline (speedup 1.0000x reference)
"""Deformable Conv2D (nn_DeformableConv2D_81810537054370) Trainium2 Bass kernel.

Sharding: 8 cores = 4 batches x 2 groups (one (b, g) shard per core, zero
cross-core communication). Per core: offset conv (PE), bilinear index/weight
math (DVE), bilinear sampling via SWDGE dma_gather, combine (DVE), PE
transposes, folded depthwise+pointwise conv (PE).

Pixel permutation: within a 128-px image row, pixel px = 8*b + u lives on
gather-out partition pg = 16*u + b. This makes every idx-scatter DMA
expressible in <=3 dims with contiguous final dims. The permutation is
undone by the access patterns of the post-transpose copies.

Self-contained: hardcodes shapes; host prep is data-independent only.
"""

import sys

for _p in ("/opt/trn_rl_repo", "/root/.axon_site/_ro/trn_rl_repo"):
    if _p not in sys.path:
        sys.path.insert(0, _p)

import numpy as np
import ml_dtypes

import concourse.bass as bass
import concourse.mybir as mybir
import concourse.tile as tile
from concourse import bacc
from concourse.masks import make_identity

F32 = mybir.dt.float32
BF16 = mybir.dt.bfloat16
I16 = mybir.dt.int16
OP = mybir.AluOpType
AF = mybir.ActivationFunctionType

# problem constants
B, H, W, C = 4, 128, 128, 128
G = 2
Cg = C // G          # 64
K2 = 9
Kin = K2 * Cg        # 576
Fg = 64
NROW = H * W         # 16384 gather rows (x'-major, y within; 4-corner rows)
NPIX = H * W
STR = 16             # output rows per stripe
NSTRIPE = H // STR   # 8
WR = STR + 2         # sampled-row window per stripe (halo)
NI = 3 * WR * 128    # idxs per gather instr (3 taps) = 6912
SLOTS_I = NI // 16   # 432
SLOTS_S = 3 * SLOTS_I            # 1296 per stripe
SLOTS_T = NSTRIPE * SLOTS_S      # 10368 per corner
NCHUNK = 5           # 576 -> 5 chunks of 128 (last zero-padded)
NF = K2 * H          # 1152


def _build_program(debug=False):
    nc = bacc.Bacc("TRN2", target_bir_lowering=False, debug=False,
                   enable_asserts=False)
    dbg = {}
    with tile.TileContext(nc) as tc:
        with tc.tile_pool(name="dram", bufs=1, space="DRAM") as dram:
            xT_d = dram.tile([Cg, H + 2, W + 2], F32, kind="ExternalInput")
            xg_d = dram.tile([NROW, 4 * Cg], BF16, kind="ExternalInput")
            b0_d = dram.tile([2 * K2, NPIX], F32, kind="ExternalInput")
            offw_d = dram.tile([128, 6 * 2 * K2], F32, kind="ExternalInput")
            wd_d = dram.tile([128, K2 * NCHUNK * Fg], BF16, kind="ExternalInput")
            bfin_d = dram.tile([Fg, 1], F32, kind="ExternalInput")
            out_d = dram.tile([Fg, NPIX], F32, kind="ExternalOutput")
            locx_d = dram.tile([K2 * NPIX], F32)   # [k][px][py]
            locy_d = dram.tile([K2 * NPIX], F32)
            topT_d = dram.tile([K2, H, 128], I16)  # [k][py][px]
            if debug:
                dbg["loc"] = dram.tile([2 * K2, NPIX], F32, kind="ExternalOutput",
                                       name="dbg_loc")
                dbg["samp"] = dram.tile([128, NSTRIPE * WR * K2 * Cg], BF16,
                                        kind="ExternalOutput", name="dbg_samp")

            with tc.tile_pool(name="persist", bufs=1) as pp, \
                 tc.tile_pool(name="pidx", bufs=1) as pidx:
                topw = pidx.tile([128, SLOTS_T], I16)
                nc.vector.memset(topw[:], 0)
                wx0b = pp.tile([128, 1154], BF16)
                wx1b = pp.tile([128, 1154], BF16)
                wy0b = pp.tile([128, 1154], BF16)
                wy1b = pp.tile([128, 1154], BF16)
                wd_sb = pp.tile([128, K2 * NCHUNK * Fg], BF16)
                bfin = pp.tile([Fg, 1], F32)
                identb = pp.tile([128, 128], BF16)
                identf = pp.tile([128, 128], F32)

                nc.sync.dma_start(wd_sb[:], wd_d[:])
                nc.sync.dma_start(bfin[:], bfin_d[:])
                make_identity(nc, identb[:])
                make_identity(nc, identf[:])
                for wt in (wx0b, wx1b, wy0b, wy1b):
                    nc.vector.memset(wt[:, 0:1], 0.0)
                    nc.vector.memset(wt[:, 1153:1154], 0.0)

                # ---- phase 1: offset conv; locA free dim is px-major ----
                # ---- phase 2: bilinear math in pg-permuted partitions ----
                with tc.tile_pool(name="ph1", bufs=1) as p1x:
                    locA = p1x.tile([2 * K2, NPIX], F32)
                    with tc.tile_pool(name="ph1a", bufs=1) as p1a, \
                         tc.tile_pool(name="ph1b", bufs=2) as p1b, \
                         tc.tile_pool(name="ps1", bufs=2, space="PSUM") as ps1:
                      # xT2: partitions 0-63 = image; 64-127 = x+1-shifted
                      # copy, so one matmul contracts taps (dy,0)+(dy,1).
                      xT = p1a.tile([128, H + 2, W + 2], F32)
                      offw = p1a.tile([128, 6 * 2 * K2], F32)
                      nc.sync.dma_start(xT[0:Cg], xT_d[:])
                      nc.sync.dma_start(offw[:], offw_d[:])
                      nc.vector.memset(xT[Cg:128, :, W + 1:W + 2], 0.0)
                      nc.sync.dma_start(xT[Cg:128, :, 0:W + 1],
                                        xT[0:Cg, :, 1:W + 2])
                      for t in range(NPIX // 512):   # 4 px-columns per tile
                        c0 = t * 4
                        b0t = p1b.tile([2 * K2, 512], F32, tag="b0t")
                        nc.sync.dma_start(b0t[:], b0_d[:, t * 512:(t + 1) * 512])
                        pt = ps1.tile([2 * K2, 512], F32, space="PSUM")
                        for j in range(6):
                            dy, dx = (j, 0) if j < 3 else (j - 3, 2)
                            rhs = xT[:, dy:dy + H,
                                     c0 + dx:c0 + dx + 4].rearrange(
                                         "c y x -> c x y")
                            nc.tensor.matmul(
                                out=pt[:],
                                lhsT=offw[:, j * 18:(j + 1) * 18],
                                rhs=rhs,
                                start=(j == 0), stop=(j == 5))
                        nc.vector.tensor_tensor(
                            out=locA[:, t * 512:(t + 1) * 512],
                            in0=pt[:], in1=b0t[:], op=OP.add)
                    if debug:
                        nc.sync.dma_start(dbg["loc"][:], locA[:])
                    # bounce locA through DRAM ([k][px][py])
                    nc.sync.dma_start(
                        locx_d[:].rearrange("(a b) -> a b", a=K2), locA[0:K2, :])
                    nc.sync.dma_start(
                        locy_d[:].rearrange("(a b) -> a b", a=K2),
                        locA[K2:2 * K2, :])

                with tc.tile_pool(name="ph2", bufs=1) as p1:
                    # phase 2 tiles (pg-partition order, free = (k, py))
                    locx = p1.tile([128, NF], F32)
                    locy = p1.tile([128, NF], F32)
                    nc.vector.memset(locx[:], 0.0)
                    nc.vector.memset(locy[:], 0.0)
                    # load in pg-partition order: partitions pg=16u+b,
                    # free (k, py); src px = 8b+u.
                    for (dst, src_d) in ((locx, locx_d), (locy, locy_d)):
                        sv = src_d[:].rearrange("(k x y) -> x k y", k=K2, x=W)
                        for u in range(8):
                            sap = sv[u::8]       # px = 8b+u, b=0..15
                            dd = dst[16 * u:16 * (u + 1), :].rearrange(
                                "p (k y) -> p k y", k=K2)
                            nc.sync.dma_start(dd, sap)

                    fr = p1.tile([128, NF], F32)
                    x0f = p1.tile([128, NF], F32)
                    x1f = p1.tile([128, NF], F32)
                    y0f = p1.tile([128, NF], F32)
                    y1f = p1.tile([128, NF], F32)
                    topf = p1.tile([128, NF], F32)

                    for loc, c0f, c1f, w0, w1 in (
                            (locx, x0f, x1f, wx0b, wx1b),
                            (locy, y0f, y1f, wy0b, wy1b)):
                        nc.vector.tensor_scalar(out=loc[:], in0=loc[:],
                                                scalar1=0.0, scalar2=float(W - 1),
                                                op0=OP.max, op1=OP.min)
                        # exact floor: r = round(loc) via 2^23 trick,
                        # then subtract 1 where r > loc
                        nc.vector.tensor_scalar(out=c0f[:], in0=loc[:],
                                                scalar1=8388608.0,
                                                scalar2=-8388608.0,
                                                op0=OP.add, op1=OP.add)
                        nc.vector.tensor_tensor(out=fr[:], in0=c0f[:],
                                                in1=loc[:], op=OP.is_gt)
                        nc.vector.tensor_sub(out=c0f[:], in0=c0f[:], in1=fr[:])
                        nc.vector.tensor_scalar(out=c1f[:], in0=c0f[:],
                                                scalar1=1.0, scalar2=float(W - 1),
                                                op0=OP.add, op1=OP.min)
                        nc.vector.tensor_sub(out=w0[:, 1:1153], in0=c1f[:],
                                             in1=loc[:])
                        nc.vector.tensor_sub(out=w1[:, 1:1153], in0=loc[:],
                                             in1=c0f[:])

                    # 4-corner row index: idx = x0*128 + y0 (y-major table)
                    nc.vector.scalar_tensor_tensor(
                        out=topf[:], in0=x0f[:], scalar=float(H), in1=y0f[:],
                        op0=OP.mult, op1=OP.add)

                    # transpose each k-block to [py, px-natural] int16, then
                    # scatter into wrapped idx layout.
                    with tc.tile_pool(name="tpi", bufs=1) as tpi, \
                         tc.tile_pool(name="pst2", bufs=4, space="PSUM") as pst2:
                        for ci, (srcf, dsti) in enumerate(
                                ((topf, topw),)):
                            tT = [tpi.tile([128, 128], I16, name=f"tT{ci}_{k}")
                                  for k in range(K2)]
                            for k in range(K2):
                                ptr = pst2.tile([128, 128], F32, space="PSUM",
                                                tag="ptr")
                                nc.tensor.transpose(
                                    out=ptr[:],
                                    in_=srcf[:, k * H:(k + 1) * H],
                                    identity=identf[:])
                                # un-permute pg -> px while casting to int16
                                src = ptr[:].rearrange("p (u b) -> p u b", u=8)
                                dd = tT[k][:].rearrange("p (b u) -> p u b",
                                                        b=16)
                                nc.scalar.copy(out=dd, in_=src)
                            # bounce tT through DRAM [k][py][px], then
                            # scatter per (s, k) into the wrapped layout
                            tT_d = topT_d
                            for k in range(K2):
                                nc.sync.dma_start(tT_d[k, :, :], tT[k][:])
                            for k in range(K2):
                                g3, kl = k // 3, k % 3

                                def sc(s, w_lo, w_hi, py0, k=k, g3=g3, kl=kl):
                                    cnt = w_hi - w_lo
                                    src = tT_d[k, py0:py0 + cnt, :].rearrange(
                                        "w (b u) -> b w u", b=16)
                                    o0 = s * SLOTS_S + g3 * 432 + kl * 144 + \
                                        8 * w_lo
                                    dd = dsti[0:16, o0:o0 + cnt * 8].rearrange(
                                        "p (w u) -> p w u", u=8)
                                    nc.sync.dma_start(dd, src)

                                for s in range(NSTRIPE):
                                    if s == 0:
                                        sc(s, 0, 1, 0)
                                        sc(s, 1, WR, 0)
                                    elif s == NSTRIPE - 1:
                                        sc(s, 0, WR - 1, STR * s - 1)
                                        sc(s, WR - 1, WR, H - 1)
                                    else:
                                        sc(s, 0, WR, STR * s - 1)
                            for a in range(1, 8):
                                nc.sync.dma_start(dsti[16 * a:16 * (a + 1), :],
                                                  dsti[0:16, :])

                # ---- phase 3: gather / combine / transpose / dwpw ----
                with tc.tile_pool(name="gb", bufs=4) as gbp, \
                     tc.tile_pool(name="cmb", bufs=3) as cmb, \
                     tc.tile_pool(name="samp", bufs=1) as smp, \
                     tc.tile_pool(name="outp", bufs=2) as outp, \
                     tc.tile_pool(name="pst", bufs=4, space="PSUM") as pst, \
                     tc.tile_pool(name="psm", bufs=1, space="PSUM") as psm:
                    samp = smp.tile([128, WR, K2, Cg], BF16)
                    sampT = [smp.tile([128, WR, W + 2], BF16, name=f"sampT{i}")
                             for i in range(NCHUNK)]
                    for i in range(NCHUNK):
                        nc.vector.memset(sampT[i][:, :, 0:1], 0.0)
                        nc.vector.memset(sampT[i][:, :, W + 1:W + 2], 0.0)
                    nc.vector.memset(sampT[4][64:128, :, :], 0.0)

                    import os as _os
                    _ns = int(_os.environ.get("KSTRIPES", NSTRIPE))
                    for s in range(_ns):
                        for k in range(K2):
                            g3, kl = k // 3, k % 3
                            gbt = gbp.tile([128, WR, 4 * Cg], BF16, tag="gt")
                            off = s * SLOTS_S + g3 * SLOTS_I + kl * 144
                            nc.gpsimd.dma_gather(
                                out_ap=gbt[:],
                                in_ap=xg_d[:],
                                idxs_ap=topw[:, off:off + 144],
                                num_idxs=2304, num_idxs_reg=2304,
                                elem_size=4 * Cg, single_packet=False)
                            if int(_os.environ.get("KPARTS", 4)) < 2:
                                continue
                            if True:
                                a_ = gbt[:, :, 0:Cg]
                                c_ = gbt[:, :, Cg:2 * Cg]
                                b_ = gbt[:, :, 2 * Cg:3 * Cg]
                                d_ = gbt[:, :, 3 * Cg:4 * Cg]
                                wsl = slice(k * H + STR * s, k * H + STR * s + WR)
                                wx0 = wx0b[:, wsl].to_broadcast([128, WR, Cg])
                                wx1 = wx1b[:, wsl].to_broadcast([128, WR, Cg])
                                wy0 = wy0b[:, wsl].to_broadcast([128, WR, Cg])
                                wy1 = wy1b[:, wsl].to_broadcast([128, WR, Cg])
                                t1 = cmb.tile([128, WR, Cg], BF16, tag="t1")
                                t2 = cmb.tile([128, WR, Cg], BF16, tag="t2")
                                t3 = cmb.tile([128, WR, Cg], BF16, tag="t3")
                                nc.vector.tensor_tensor(out=t1[:], in0=a_, in1=wx0, op=OP.mult)
                                nc.vector.tensor_tensor(out=t2[:], in0=c_, in1=wx1, op=OP.mult)
                                nc.vector.tensor_tensor(out=t1[:], in0=t1[:], in1=t2[:], op=OP.add)
                                nc.vector.tensor_tensor(out=t2[:], in0=b_, in1=wx0, op=OP.mult)
                                nc.vector.tensor_tensor(out=t3[:], in0=d_, in1=wx1, op=OP.mult)
                                nc.vector.tensor_tensor(out=t2[:], in0=t2[:], in1=t3[:], op=OP.add)
                                nc.vector.tensor_tensor(out=t1[:], in0=t1[:], in1=wy0, op=OP.mult)
                                nc.vector.tensor_tensor(out=t2[:], in0=t2[:], in1=wy1, op=OP.mult)
                                nc.vector.tensor_tensor(
                                    out=samp[:, :, k, :], in0=t1[:], in1=t2[:], op=OP.add)
                        if debug:
                            nc.sync.dma_start(
                                dbg["samp"][:, s * WR * Kin:(s + 1) * WR * Kin],
                                samp[:].rearrange("p a b c -> p (a b c)"))
                        if int(_os.environ.get("KPARTS", 4)) < 3:
                            continue
                        # transposes into sampT (un-permuting pg -> px)
                        w_lo = 1 if s == 0 else 0
                        w_hi = WR - 1 if s == NSTRIPE - 1 else WR
                        if s == 0:
                            for i in range(NCHUNK):
                                nc.vector.memset(sampT[i][:, 0, :], 0.0)
                        if s == NSTRIPE - 1:
                            for i in range(NCHUNK):
                                nc.vector.memset(sampT[i][:, WR - 1, :], 0.0)
                        for wrow in range(w_lo, w_hi):
                            for kp in range(NCHUNK):
                                kk = 2 * kp
                                width = 128 if kp < 4 else 64
                                src = samp[:, wrow, kk:kk + (2 if kp < 4 else 1), :]
                                ptt = pst.tile([128, 128], BF16, space="PSUM",
                                               tag="ptt")
                                nc.tensor.transpose(
                                    out=ptt[:width, :],
                                    in_=src.rearrange("p a b -> p (a b)"),
                                    identity=identb[:])
                                src2 = ptt[:width, :].rearrange(
                                    "p (u b) -> p u b", u=8)
                                dd = sampT[kp][:width, wrow, 1:1 + W].rearrange(
                                    "p (b u) -> p u b", b=16)
                                nc.scalar.copy(out=dd, in_=src2)
                        if int(_os.environ.get("KPARTS", 4)) < 4:
                            continue
                        # dwpw matmuls: (d, ci) outer so one weight feeds all
                        # 4 px-tiles back-to-back (denser PE, fewer reloads)
                        pms = [psm.tile([Fg, 512], F32, space="PSUM",
                                        tag=f"pm{t}", name=f"pm{t}_{s}")
                               for t in range(4)]
                        for dy in (-1, 0, 1):
                            for dx in (-1, 0, 1):
                                d_i = (dy + 1) * 3 + (dx + 1)
                                for ci in range(NCHUNK):
                                    lhs = wd_sb[:, (d_i * NCHUNK + ci) * Fg:
                                                (d_i * NCHUNK + ci + 1) * Fg]
                                    first = (dy == -1 and dx == -1 and ci == 0)
                                    last = (dy == 1 and dx == 1 and
                                            ci == NCHUNK - 1)
                                    for t in range(4):
                                        wr0 = t * 4 + 1 + dy
                                        rhs = sampT[ci][:, wr0:wr0 + 4,
                                                        1 + dx:1 + dx + W]
                                        nc.tensor.matmul(out=pms[t][:],
                                                         lhsT=lhs, rhs=rhs,
                                                         start=first,
                                                         stop=last)
                        for t in range(4):
                            ot = outp.tile([Fg, 512], F32, tag="ot")
                            nc.scalar.activation(out=ot[:], in_=pms[t][:],
                                                 func=AF.Identity, bias=bfin[:],
                                                 scale=1.0)
                            nc.sync.dma_start(
                                out_d[:, s * 2048 + t * 512:
                                      s * 2048 + (t + 1) * 512],
                                ot[:])
    nc.compile()
    names = dict(xT=xT_d.name, xg=xg_d.name, b0=b0_d.name, offw=offw_d.name,
                 wd=wd_d.name, bfin=bfin_d.name, out=out_d.name,
                 dbg={k: v.name for k, v in dbg.items()})
    return nc, names


def _host_prep(x, off_w, off_b, dw_w, dw_b, pw_w, pw_b, b, g):
    """Data-independent prep of one (b, g) shard's device inputs."""
    xi = np.asarray(x)[b, :, :, g * Cg:(g + 1) * Cg].astype(np.float32)
    xT = np.zeros((Cg, H + 2, W + 2), np.float32)
    xT[:, 1:H + 1, 1:W + 1] = xi.transpose(2, 0, 1)
    # 4-corner gather rows, x'-major: row x0*H + y0 =
    # [x(y0,x0), x(y0,x0+1), x(y0+1,x0), x(y0+1,x0+1)] (edge-clamped)
    xp = np.pad(xi, ((0, 1), (0, 1), (0, 0)), mode="edge")
    top = np.concatenate([xp[:, :W], xp[:, 1:W + 1]], axis=2)
    pairs = np.concatenate([top[:-1], top[1:]], axis=2)
    xg = pairs.transpose(1, 0, 2).reshape(NROW, 4 * Cg).astype(
        ml_dtypes.bfloat16)
    # base tables [18, NPIX], free dim px-major (px*H + py)
    lin = np.array([-1.0, 0.0, 1.0], np.float32)
    gx, gy = np.meshgrid(np.arange(W, dtype=np.float32),
                         np.arange(H, dtype=np.float32))
    gxT, gyT = gx.T.reshape(-1), gy.T.reshape(-1)   # px-major flatten
    ob = np.asarray(off_b)[g].astype(np.float32)
    b0 = np.zeros((2 * K2, NPIX), np.float32)
    for k in range(K2):
        b0[k] = gxT + lin[k % 3] + ob[2 * k]
        b0[K2 + k] = gyT + lin[k // 3] + ob[2 * k + 1]
    ow = np.asarray(off_w)[g].astype(np.float32)
    # packed: j<3 pairs taps (j,0)@rows0-63 + (j,1)@rows64-127; j>=3 is
    # single tap (j-3,2)@rows0-63 (rows 64-127 zero)
    offw = np.zeros((128, 6 * 2 * K2), np.float32)
    for j in range(6):
        lo = ow[j, 0] if j < 3 else ow[j - 3, 2]
        offw[0:Cg, j * 18:j * 18 + K2] = lo[:, 0::2]
        offw[0:Cg, j * 18 + K2:(j + 1) * 18] = lo[:, 1::2]
        if j < 3:
            hi = ow[j, 1]
            offw[Cg:128, j * 18:j * 18 + K2] = hi[:, 0::2]
            offw[Cg:128, j * 18 + K2:(j + 1) * 18] = hi[:, 1::2]
    dw = np.asarray(dw_w)[g, :, :, 0, :].astype(np.float32)
    pw = np.asarray(pw_w)[g, 0, 0].astype(np.float32)
    wd = np.zeros((128, K2 * NCHUNK * Fg), np.float32)
    for d_i in range(K2):
        wfull = dw[d_i // 3, d_i % 3][:, None] * pw
        for ci in range(NCHUNK):
            rows = min(128, Kin - ci * 128)
            wd[:rows, (d_i * NCHUNK + ci) * Fg:(d_i * NCHUNK + ci + 1) * Fg] = \
                wfull[ci * 128:ci * 128 + rows]
    wd = wd.astype(ml_dtypes.bfloat16)
    bfin = (pw.T @ np.asarray(dw_b)[g].astype(np.float32)
            + np.asarray(pw_b)[g].astype(np.float32)).reshape(Fg, 1)
    return dict(xT=xT, xg=xg, b0=b0, offw=offw, wd=wd, bfin=bfin)


_CACHE = {}


def _get_program(debug=False):
    key = ("prog", debug)
    if key not in _CACHE:
        _CACHE[key] = _build_program(debug=debug)
    return _CACHE[key]


def kernel(x, off_w, off_b, dw_w, dw_b, pw_w, pw_b):
    from concourse import bass_utils
    nc, names = _get_program()
    shards = [(b, g) for b in range(B) for g in range(G)]
    in_maps = []
    for b, g in shards:
        prep = _host_prep(x, off_w, off_b, dw_w, dw_b, pw_w, pw_b, b, g)
        in_maps.append({names[k]: v for k, v in prep.items()})
    res = bass_utils.run_bass_kernel_spmd(nc, in_maps, core_ids=list(range(8)))
    out = np.zeros((B, H, W, C), np.float32)
    for i, (b, g) in enumerate(shards):
        o = np.asarray(res.results[i][names["out"]])  # [Fg, NPIX]
        out[b, :, :, g * Cg:(g + 1) * Cg] = \
            o.reshape(Fg, H, W).transpose(1, 2, 0)
    return out

